# revision 20
# baseline (speedup 1.0000x reference)
"""Multi-head attention Trainium2 Bass kernel (8 NeuronCores).

Problem: B=2, S=2048, D=1024, H=16, Dh=64, scale=1/sqrt(D).
Sharding: batch x head. Core c handles batch c//4, heads (c%4)*4 .. +4.
No collectives: per-core outputs carry fused softmax denominators (row 64
of each head's projected tile); the host divides, sums heads and adds b_o.

Design notes (v3):
  - ScalarE exp is the largest single-engine load (~118us/core); PE and
    DVE are close behind, so the schedule overlaps all three:
      k-proj head-start -> per-qg loop where ctx/proj of the current
      group and q-proj of the next interleave as "filler" between the
      pps groups of the NEXT group's logits (the pps ring paces PE to
      ACT speed; filler runs in the gaps instead of stalling in-order PE).
  - Everything stored bf16 (x, W, q, k, v, P, mask, ctx, outputs); PSUM
    accumulation stays fp32. rel-err budget ~1e-2 >> bf16 noise.
  - exp tiles are [128, 1536] (3 PSUM banks x 2 bufs) to amortize the
    ~352-cycle ACT per-instruction overhead.
  - logits head pairs use disjoint PE row groups (partitions 0-63/64-127)
    and run concurrently on HW.
  - proj matmul is augmented to [65,65] per head: row 64 passes the
    softmax denominator through, so one output DMA carries both.
PSUM budget: pps 2x3 banks + misc 1 (ctx/po ring) + proj 1 (k/v/q-proj) = 8.
"""

import numpy as np
import ml_dtypes

import concourse.bass as bass  # noqa: F401
import concourse.tile as tile
from concourse import bacc, mybir
from concourse.bass_utils import run_bass_kernel_spmd

B, S, D = 2, 2048, 1024
H, Dh = 16, 64
NCORE = 8
GPB = NCORE // B            # cores per batch (4)
HL = H // GPB               # local heads per core (4)
SCALE = float(1.0 / np.sqrt(np.float32(D)))

F32 = mybir.dt.float32
BF16 = mybir.dt.bfloat16
F8 = mybir.dt.float8e4
DR = mybir.MatmulPerfMode.DoubleRow
NDP = 4   # fp8 DoubleRow d-pair chunks (each = 256 contraction)
W8SCALE = 16.0  # host premultiplier on Wq/Wk to dodge fp8 subnormals

ABLATE = set()

ND = D // 128    # 8 contraction chunks
NCH = S // 128   # 16 sk chunks
NQG = S // 512   # 4 query groups
NSL = 2 * NCH    # 32 (j,hh) 512-wide logit slices per (qg, g)


def build_module(reps=1, ablate=(), timing_mode=False, qk8=False):
    """timing_mode: big inputs (x, mask) become Internal DRAM (garbage
    contents, zero per-call upload) and all reps write one small output
    region.  On-device DMA/compute traffic is identical to the real
    module; only host<->device transfer shrinks.  Used for wall-clock
    reps-delta timing where per-call upload noise would swamp exec."""
    ablate = set(ablate)
    nc = bacc.Bacc("TRN2", target_bir_lowering=False, debug=False,
                   num_devices=NCORE)

    big = "Internal" if timing_mode else "ExternalInput"
    xT = nc.dram_tensor("xT", [D, S], BF16, kind=big).ap()
    xT8 = nc.dram_tensor("xT8", [D, S], F8, kind=big).ap()
    wqk = nc.dram_tensor("wqk", [D, 2 * HL * Dh], F8, kind="ExternalInput").ap()
    wqkb = nc.dram_tensor("wqkb", [D, 2 * HL * Dh], BF16, kind="ExternalInput").ap()
    wv = nc.dram_tensor("wv", [D, HL * Dh], BF16, kind="ExternalInput").ap()
    bqk = nc.dram_tensor("bqk", [128, 4], F32, kind="ExternalInput").ap()
    bv = nc.dram_tensor("bv", [128, HL * Dh], F32, kind="ExternalInput").ap()
    wo = nc.dram_tensor("wo", [Dh + 1, HL * (Dh + 1)], BF16,
                        kind="ExternalInput").ap()
    maskT = nc.dram_tensor("maskT", [S, S], BF16, kind=big).ap()
    nrout = 1 if timing_mode else reps
    outp = nc.dram_tensor("outp", [nrout * HL, Dh + 1, S], BF16,
                          kind="ExternalOutput").ap()

    EXP = mybir.ActivationFunctionType.Exp

    with tile.TileContext(nc) as tc:
        with (
            tc.tile_pool(name="const", bufs=1) as constp,
            tc.tile_pool(name="kpool", bufs=1) as kpoolp,
            tc.tile_pool(name="vpool", bufs=1) as vpoolp,
            tc.tile_pool(name="xpool", bufs=1) as xpoolp,
            # transient rings
            tc.tile_pool(name="qpool", bufs=6, side="right") as qpoolp,
            tc.tile_pool(name="maskp", bufs=3, side="right") as maskp,
            tc.tile_pool(name="ptp", bufs=2, side="right") as ptp,
            tc.tile_pool(name="cup", bufs=3, side="right") as cup,
            tc.tile_pool(name="pps", space="PSUM", bufs=2) as ppsp,
            tc.tile_pool(name="miscps", space="PSUM", bufs=1) as miscp,
            tc.tile_pool(name="projps", space="PSUM", bufs=1) as projp,
        ):
            # ---------------- persistent tiles ----------------
            if qk8:
                wqk_sb = constp.tile([128, 4 * NDP * 2 * 128], F8)
                nc.sync.dma_start(
                    wqk_sb.rearrange("p (b c t m) -> p b c t m",
                                     b=4, c=NDP, t=2),
                    wqk.rearrange("(c t p) (b m) -> p b c t m",
                                  p=128, t=2, b=4))
            else:
                wqk_sb = constp.tile([128, ND * 512], BF16)
                nc.sync.dma_start(
                    wqk_sb.rearrange("p (d c) -> p d c", d=ND),
                    wqkb.rearrange("(d p) c -> p d c", p=128))
            wv_sb = constp.tile([128, ND * 256], BF16)
            nc.sync.dma_start(
                wv_sb.rearrange("p (d c) -> p d c", d=ND),
                wv.rearrange("(d p) c -> p d c", p=128))
            wo_sb = constp.tile([Dh + 1, HL * (Dh + 1)], BF16)
            nc.sync.dma_start(wo_sb, wo)
            bqk_sb = constp.tile([128, 4], F32)
            nc.sync.dma_start(bqk_sb, bqk)
            bv_sb = constp.tile([128, HL * Dh], F32)
            nc.sync.dma_start(bv_sb, bv)

            # kT per head pair g: rows 0-63 head 2g, rows 64-127 head 2g+1
            kt = [kpoolp.tile([128, S], BF16, name=f"kt{g}")
                  for g in range(HL // 2)]
            # v in [sk, d] layout + fused ones column per head
            v_sb = vpoolp.tile([128, NCH * HL * 65], BF16)
            nc.vector.memset(
                v_sb.rearrange("p (m c) -> p m c", c=65)[:, :, 64:65], 1.0)
            # x^T tiles, one per 512-column block (= per query group)
            xt = [xpoolp.tile([128, ND * 512], BF16, name=f"xt{s4}")
                  for s4 in range(NQG)]
            xt8 = [xpoolp.tile([128, NDP * 2 * 512], F8, name=f"xt8{s4}")
                   for s4 in range(NQG)] if qk8 else None

            def dma_x(s4):
                if "dma_x" in ablate:
                    return
                nc.sync.dma_start(
                    xt[s4].rearrange("p (d c) -> p d c", d=ND),
                    xT.rearrange("(d p) s -> p d s", p=128)
                      [:, :, s4 * 512:(s4 + 1) * 512])
                if qk8:
                    nc.sync.dma_start(
                        xt8[s4].rearrange("p (c t s) -> p c t s",
                                          c=NDP, t=2),
                        xT8.rearrange("(c t p) s -> p c t s", p=128, t=2)
                           [:, :, :, s4 * 512:(s4 + 1) * 512])

            def kproj(s4, use_misc):
                """k-proj for x block s4: both pair-blocks -> kt slices."""
                if qk8:
                    wv8 = wqk_sb.rearrange("p (b c t m) -> p b c t m",
                                           b=4, c=NDP, t=2)
                    xv8 = xt8[s4].rearrange("p (c t s) -> p c t s",
                                            c=NDP, t=2)
                for blk in range(2):
                    pool = miscp if (use_misc ^ (blk == 0)) else projp
                    ps = pool.tile([128, 512], F32, tag="mc" if pool is miscp
                                   else "pj")
                    if qk8:
                        nlast = NDP - 1 if "qkv" not in ablate else 0
                        for c in range(nlast + 1):
                            nc.tensor.matmul(
                                ps,
                                lhsT=wv8[:, 2 + blk, c],
                                rhs=xv8[:, c],
                                start=(c == 0), stop=(c == nlast),
                                perf_mode=DR)
                    else:
                        nlast = ND - 1 if "qkv" not in ablate else 0
                        for dd in range(nlast + 1):
                            nc.tensor.matmul(
                                ps,
                                lhsT=wqk_sb[:, dd * 512 + (2 + blk) * 128:
                                            dd * 512 + (3 + blk) * 128],
                                rhs=xt[s4][:, dd * 512:(dd + 1) * 512],
                                start=(dd == 0), stop=(dd == nlast))
                    nc.vector.tensor_scalar_add(
                        kt[blk][:, s4 * 512:(s4 + 1) * 512], ps,
                        bqk_sb[:, 2 + blk:3 + blk])

            def qproj(qg):
                """q-proj for query group qg -> fresh qt tiles (per pair)."""
                qts = []
                if qk8:
                    wv8 = wqk_sb.rearrange("p (b c t m) -> p b c t m",
                                           b=4, c=NDP, t=2)
                    xv8 = xt8[qg].rearrange("p (c t s) -> p c t s",
                                            c=NDP, t=2)
                for blk in range(2):
                    ps = projp.tile([128, 512], F32, tag="pj")
                    if qk8:
                        nlast = NDP - 1 if "qkv" not in ablate else 0
                        for c in range(nlast + 1):
                            nc.tensor.matmul(
                                ps,
                                lhsT=wv8[:, blk, c],
                                rhs=xv8[:, c],
                                start=(c == 0), stop=(c == nlast),
                                perf_mode=DR)
                    else:
                        nlast = ND - 1 if "qkv" not in ablate else 0
                        for dd in range(nlast + 1):
                            nc.tensor.matmul(
                                ps,
                                lhsT=wqk_sb[:, dd * 512 + blk * 128:
                                            dd * 512 + (blk + 1) * 128],
                                rhs=xt[qg][:, dd * 512:(dd + 1) * 512],
                                start=(dd == 0), stop=(dd == nlast))
                    qt = qpoolp.tile([128, 512], BF16, tag="qt")
                    nc.vector.tensor_scalar_add(
                        qt, ps, bqk_sb[:, blk:blk + 1])
                    qts.append(qt)
                return qts

            def vproj_one(s4, jj):
                """v-proj for one sk chunk (j = s4*4 + jj) -> v_sb."""
                j = s4 * 4 + jj
                pool = miscp if j % 2 == 0 else projp
                psv = pool.tile([128, 512], F32,
                                tag="mc" if pool is miscp else "pj")
                for d in range(ND if "qkv" not in ablate else 1):
                    nc.tensor.matmul(
                        psv[:, 0:256],
                        lhsT=xt[s4][:, d * 512 + jj * 128:
                                    d * 512 + jj * 128 + 128],
                        rhs=wv_sb[:, d * 256:(d + 1) * 256],
                        start=(d == 0),
                        stop=(d == (ND - 1 if "qkv" not in ablate else 0)))
                nc.vector.tensor_add(
                    v_sb[:, j * (HL * 65):(j + 1) * (HL * 65)]
                        .rearrange("p (h c) -> p h c", h=HL)[:, :, 0:64],
                    psv[:, 0:256].rearrange("p (h c) -> p h c", h=HL),
                    bv_sb.rearrange("p (h c) -> p h c", h=HL))

            def dma_mask(qg):
                mts = []
                for r in range(2):
                    mt = maskp.tile([128, 8 * 512], BF16, name=f"mt{r}",
                                    tag=f"mask{r}")
                    if "dma_mask" not in ablate:
                        nc.sync.dma_start(
                            mt.rearrange("p (j c) -> p j c", j=8),
                            maskT.rearrange("(j p) q -> p j q", p=128)
                                 [:, 8 * r:8 * r + 8,
                                  qg * 512:(qg + 1) * 512])
                    mts.append(mt)
                return mts

            def logits(qg, g, qts, filler=None):
                """Logits + exp for head pair g of query group qg.
                Returns the pt tile [128, NSL*512] of unnormalized
                probabilities (mask applied separately).  `filler` is an
                iterator of callables emitting independent PE work between
                pps groups (the pps ring paces PE to ACT speed, so filler
                work runs in the gaps instead of stalling the PE stream)."""
                pt = ptp.tile([128, NSL * 512], BF16, tag="pt")
                s0 = 0
                pps = None
                for j in range(NCH):
                    for hh in range(2):
                        s = 2 * j + hh
                        if pps is None:
                            pps = ppsp.tile([128, 1536], F32, tag="pps")
                            s0 = s
                        if "logits" not in ablate or j == 0:
                            nc.tensor.matmul(
                                pps[:, (s - s0) * 512:(s - s0 + 1) * 512],
                                lhsT=kt[g][hh * 64:(hh + 1) * 64,
                                           j * 128:(j + 1) * 128],
                                rhs=qts[g][hh * 64:(hh + 1) * 64, :],
                                start=True, stop=True)
                        if s - s0 == 2 or s == NSL - 1:
                            if "exp" not in ablate or s0 == 0:
                                nc.scalar.activation(
                                    pt[:, s0 * 512:(s + 1) * 512],
                                    pps[:, 0:(s - s0 + 1) * 512], EXP,
                                    scale=SCALE / (W8SCALE * W8SCALE))
                            pps = None
                            if filler is not None:
                                for fn in next(filler, []):
                                    fn()
                return pt

            def mask_mul(pt, mts, r):
                if "mask" in ablate:
                    return
                ptv = pt[:, r * 16 * 512:(r + 1) * 16 * 512].rearrange(
                    "p (j e c) -> p j e c", j=8, e=2)
                mtv = mts[r].rearrange("p (j c) -> p j c", j=8)
                for e in range(2):
                    nc.vector.tensor_mul(ptv[:, :, e, :], ptv[:, :, e, :], mtv)

            def ctx_chunks(h, ctx, pt, hh, j0, j1):
                nj = NCH if "ctx" not in ablate else 1
                for j in range(j0, min(j1, nj)):
                    nc.tensor.matmul(
                        ctx,
                        lhsT=v_sb[:, j * (HL * 65) + h * 65:
                                  j * (HL * 65) + (h + 1) * 65],
                        rhs=pt[:, (2 * j + hh) * 512:
                               (2 * j + hh + 1) * 512],
                        start=(j == 0),
                        stop=(j == nj - 1),
                        skip_group_check=True)

            def ctx_proj_items(rep, qg, g, pt):
                """Generator of filler items: ctx (with fused denominator
                row) + output proj for both heads of pair g, sliced into
                small PE bursts so they interleave with logits groups."""
                for hh in range(2):
                    h = 2 * g + hh
                    ctx = miscp.tile([65, 512], F32, tag="mc")
                    for j0 in range(0, NCH, 4):
                        yield [lambda a=j0: ctx_chunks(h, ctx, pt, hh, a,
                                                       a + 4)]

                    def finish(h=h, ctx=ctx):
                        cu = cup.tile([65, 512], BF16, tag="cu")
                        nc.vector.tensor_copy(cu, ctx)
                        po = miscp.tile([65, 512], F32, tag="mc")
                        if "proj" not in ablate:
                            nc.tensor.matmul(
                                po,
                                lhsT=wo_sb[:, h * 65:(h + 1) * 65],
                                rhs=cu,
                                start=True, stop=True)
                        po_sb = cup.tile([65, 512], BF16, tag="po")
                        nc.vector.tensor_copy(po_sb, po)
                        nc.sync.dma_start(
                            outp[(rep % nrout) * HL + h]
                                [:, qg * 512:(qg + 1) * 512],
                            po_sb)
                    yield [finish]

            def ctx_proj(rep, qg, g, pt):
                for items in ctx_proj_items(rep, qg, g, pt):
                    for fn in items:
                        fn()

            # ---------------- schedule ----------------
            def vproj_groups():
                """One v-proj accumulation group (one sk chunk) at a time,
                ping-ponged across the two spare PSUM banks."""
                for s4 in range(NQG):
                    for jj in range(4):
                        yield [lambda s4=s4, jj=jj: vproj_one(s4, jj)]

            for rep in range(reps):
                for s4 in range(NQG):
                    dma_x(s4)
                # head start: k proj; then attention with v-proj slotted
                # into the pps-ring gaps of the first logits call.
                for s4 in range(NQG):
                    kproj(s4, use_misc=(s4 % 2 == 0))
                mts = dma_mask(0)
                qts = qproj(0)
                vfill = vproj_groups()
                pt00 = logits(0, 0, qts, filler=vfill)
                mask_mul(pt00, mts, 0)
                pt01 = logits(0, 1, qts, filler=vfill)
                mask_mul(pt00, mts, 1)
                qts_n = qproj(1)

                pts = [pt00, pt01]
                for qg in range(NQG):
                    # pt tiles of current qg are in pts; qts_n = q of qg+1
                    mts_n = dma_mask(qg + 1) if qg + 1 < NQG else None
                    if qg + 1 < NQG:
                        # ctx/proj of (qg, g) interleave as filler inside
                        # logits(qg+1, g) so the pps ring keeps ACT fed.
                        fill0 = ctx_proj_items(rep, qg, 0, pts[0])
                        n0 = logits(qg + 1, 0, qts_n, filler=fill0)
                        for items in fill0:
                            for fn in items:
                                fn()
                        mask_mul(pts[1], mts, 0)
                        mask_mul(pts[1], mts, 1)

                        qts_nn = []

                        def fill1_gen(qg=qg, pts=pts, qts_nn=qts_nn):
                            yield from ctx_proj_items(rep, qg, 1, pts[1])
                            if qg + 2 < NQG:
                                yield [lambda: qts_nn.extend(qproj(qg + 2))]

                        fill1 = fill1_gen()
                        n1 = logits(qg + 1, 1, qts_n, filler=fill1)
                        for items in fill1:
                            for fn in items:
                                fn()
                        mask_mul(n0, mts_n, 0)
                        mask_mul(n0, mts_n, 1)
                        pts = [n0, n1]
                        mts = mts_n
                        if qts_nn:
                            qts_n = qts_nn
                    else:
                        ctx_proj(rep, qg, 0, pts[0])
                        mask_mul(pts[1], mts, 0)
                        mask_mul(pts[1], mts, 1)
                        ctx_proj(rep, qg, 1, pts[1])

    nc.compile()
    return nc


_NC_CACHE = {}


def get_module(reps=1, timing_mode=False, ablate=(), qk8=False):
    key = (reps, timing_mode, tuple(sorted(ablate)), qk8)
    if key not in _NC_CACHE:
        _NC_CACHE[key] = build_module(reps, ablate=ablate,
                                      timing_mode=timing_mode, qk8=qk8)
    return _NC_CACHE[key]


def make_in_maps(x, W_qkv, b_qkv, W_o, b_o, mask):
    x = np.asarray(x, np.float32)
    W_qkv = np.asarray(W_qkv, np.float32)
    b_qkv = np.asarray(b_qkv, np.float32)
    W_o = np.asarray(W_o, np.float32)
    mask = np.asarray(mask)
    BF = ml_dtypes.bfloat16

    # reference layout: W_qkv[:, h*3*Dh + {0..Dh | Dh..2Dh | 2Dh..3Dh}] =
    # q|k|v of head h (qkv.reshape(B,S,H,3*Dh) then split on last axis)
    W3 = W_qkv.reshape(D, H, 3 * Dh)
    b3 = b_qkv.reshape(H, 3 * Dh)
    Wq = np.ascontiguousarray(W3[:, :, :Dh].reshape(D, H * Dh))
    Wk = np.ascontiguousarray(W3[:, :, Dh:2 * Dh].reshape(D, H * Dh))
    Wv = np.ascontiguousarray(W3[:, :, 2 * Dh:].reshape(D, H * Dh))
    bq = np.ascontiguousarray(b3[:, :Dh].reshape(H * Dh))
    bk = np.ascontiguousarray(b3[:, Dh:2 * Dh].reshape(H * Dh))
    bv_full = np.ascontiguousarray(b3[:, 2 * Dh:].reshape(H * Dh))

    F8NP = ml_dtypes.float8_e4m3
    xT_b = [np.ascontiguousarray(x[b].T).astype(BF) for b in range(B)]
    xT8_b = [np.ascontiguousarray(x[b].T).astype(F8NP) for b in range(B)]
    maskT_b = [np.ascontiguousarray(
        (mask[b, 0] != 0).T.astype(BF)) for b in range(B)]

    in_maps = []
    for c in range(NCORE):
        b = c // GPB
        g0 = (c % GPB) * HL  # first global head of this core
        # pair-blocks: [q(2g0..), q(..), k(..), k(..)] each 128 cols.
        # fp8 path: x16 premultiplier keeps W out of fp8 subnormals; the
        # 1/sqrt(D) scale and the 16*16 factor are folded into exp's scale.
        qcols = [Wq[:, (g0 + 2 * p) * 64:(g0 + 2 * p + 2) * 64] * W8SCALE
                 for p in range(HL // 2)]
        kcols = [Wk[:, (g0 + 2 * p) * 64:(g0 + 2 * p + 2) * 64] * W8SCALE
                 for p in range(HL // 2)]
        wqk_c = np.ascontiguousarray(np.concatenate(qcols + kcols, axis=1))
        wv_c = np.ascontiguousarray(Wv[:, g0 * 64:(g0 + HL) * 64])
        bqk_c = np.stack(
            [bq[(g0 + 2 * p) * 64:(g0 + 2 * p + 2) * 64] * W8SCALE
             for p in range(HL // 2)]
            + [bk[(g0 + 2 * p) * 64:(g0 + 2 * p + 2) * 64] * W8SCALE
               for p in range(HL // 2)], axis=1)
        bv_c = np.tile(bv_full[g0 * 64:(g0 + HL) * 64], (128, 1))
        # augmented per-head proj: [65, 65] with denominator pass-through
        wo_c = np.zeros((Dh + 1, HL * (Dh + 1)), np.float32)
        for h in range(HL):
            wo_c[0:Dh, h * 65:h * 65 + Dh] = W_o[(g0 + h) * 64:
                                                 (g0 + h + 1) * 64, :]
            wo_c[Dh, h * 65 + Dh] = 1.0
        in_maps.append({
            "xT": xT_b[b],
            "xT8": xT8_b[b],
            "wqk": wqk_c.astype(F8NP),
            "wqkb": wqk_c.astype(BF),
            "wv": wv_c.astype(BF),
            "bqk": np.ascontiguousarray(bqk_c, dtype=np.float32),
            "bv": np.ascontiguousarray(bv_c, dtype=np.float32),
            "wo": wo_c.astype(BF),
            "maskT": maskT_b[b],
        })
    return in_maps


def combine_outputs(results, b_o):
    """results: list of 8 dicts with 'outp' [HL, Dh+1, S] (bf16)."""
    b_o = np.asarray(b_o, np.float32)
    out = np.zeros((B, S, Dh), np.float32)
    for c in range(NCORE):
        b = c // GPB
        op = results[c]["outp"].astype(np.float32)     # [HL, 65, S]
        contrib = (op[:, :Dh, :] / op[:, Dh:Dh + 1, :]).sum(axis=0)
        out[b] += contrib.T
    out += b_o[None, None, :]
    return out


def kernel(x, W_qkv, b_qkv, W_o, b_o, mask):
    nc = get_module()
    in_maps = make_in_maps(x, W_qkv, b_qkv, W_o, b_o, mask)
    res = run_bass_kernel_spmd(nc, in_maps, core_ids=list(range(NCORE)))
    return combine_outputs(res.results, b_o)


# revision 22
# speedup vs baseline: 1.2975x; 1.2975x over previous
"""Multi-head attention Trainium2 Bass kernel (8 NeuronCores).

Problem: B=2, S=2048, D=1024, H=16, Dh=64, scale=1/sqrt(D).
Sharding: batch x head. Core c handles batch c//4, heads (c%4)*4 .. +4.
No collectives: per-core outputs carry fused softmax denominators (row 64
of each head's projected tile); the host divides, sums heads and adds b_o.

Design notes (v3):
  - ScalarE exp is the largest single-engine load (~118us/core); PE and
    DVE are close behind, so the schedule overlaps all three:
      k-proj head-start -> per-qg loop where ctx/proj of the current
      group and q-proj of the next interleave as "filler" between the
      pps groups of the NEXT group's logits (the pps ring paces PE to
      ACT speed; filler runs in the gaps instead of stalling in-order PE).
  - Everything stored bf16 (x, W, q, k, v, P, mask, ctx, outputs); PSUM
    accumulation stays fp32. rel-err budget ~1e-2 >> bf16 noise.
  - exp tiles are [128, 1024] in a depth-3 PSUM ring (2 banks x 3 bufs):
    ring depth beats exp-tile width here -- phase-boundary handoff
    latency, not ACT instruction overhead, is the binding effect.
  - logits head pairs use disjoint PE row groups (partitions 0-63/64-127)
    and run concurrently on HW.
  - proj matmul is augmented to [65,65] per head: row 64 passes the
    softmax denominator through, so one output DMA carries both.
PSUM budget: pps 3x2 banks + misc 1 (ctx/po ring) + proj 1 (k/v/q-proj) = 8.
"""

import numpy as np
import ml_dtypes

import concourse.bass as bass  # noqa: F401
import concourse.tile as tile
from concourse import bacc, mybir
from concourse.bass_utils import run_bass_kernel_spmd

B, S, D = 2, 2048, 1024
H, Dh = 16, 64
NCORE = 8
GPB = NCORE // B            # cores per batch (4)
HL = H // GPB               # local heads per core (4)
SCALE = float(1.0 / np.sqrt(np.float32(D)))

F32 = mybir.dt.float32
BF16 = mybir.dt.bfloat16
F8 = mybir.dt.float8e4
DR = mybir.MatmulPerfMode.DoubleRow
NDP = 4   # fp8 DoubleRow d-pair chunks (each = 256 contraction)
W8SCALE = 16.0  # host premultiplier on Wq/Wk to dodge fp8 subnormals

ABLATE = set()

ND = D // 128    # 8 contraction chunks
NCH = S // 128   # 16 sk chunks
NQG = S // 512   # 4 query groups
NSL = 2 * NCH    # 32 (j,hh) 512-wide logit slices per (qg, g)


def build_module(reps=1, ablate=(), timing_mode=False, qk8=False,
                 gw=2, ppsbufs=3):
    """timing_mode: big inputs (x, mask) become Internal DRAM (garbage
    contents, zero per-call upload) and all reps write one small output
    region.  On-device DMA/compute traffic is identical to the real
    module; only host<->device transfer shrinks.  Used for wall-clock
    reps-delta timing where per-call upload noise would swamp exec."""
    ablate = set(ablate)
    nc = bacc.Bacc("TRN2", target_bir_lowering=False, debug=False,
                   num_devices=NCORE)

    big = "Internal" if timing_mode else "ExternalInput"
    xT = nc.dram_tensor("xT", [D, S], BF16, kind=big).ap()
    xT8 = nc.dram_tensor("xT8", [D, S], F8, kind=big).ap()
    wqk = nc.dram_tensor("wqk", [D, 2 * HL * Dh], F8, kind="ExternalInput").ap()
    wqkb = nc.dram_tensor("wqkb", [D, 2 * HL * Dh], BF16, kind="ExternalInput").ap()
    wv = nc.dram_tensor("wv", [D, HL * Dh], BF16, kind="ExternalInput").ap()
    bqk = nc.dram_tensor("bqk", [128, 4], F32, kind="ExternalInput").ap()
    bv = nc.dram_tensor("bv", [128, HL * Dh], F32, kind="ExternalInput").ap()
    wo = nc.dram_tensor("wo", [Dh + 1, HL * (Dh + 1)], BF16,
                        kind="ExternalInput").ap()
    maskT = nc.dram_tensor("maskT", [S, S], BF16, kind=big).ap()
    nrout = 1 if timing_mode else reps
    outp = nc.dram_tensor("outp", [nrout * HL, Dh + 1, S], BF16,
                          kind="ExternalOutput").ap()

    EXP = mybir.ActivationFunctionType.Exp

    with tile.TileContext(nc) as tc:
        with (
            tc.tile_pool(name="const", bufs=1) as constp,
            tc.tile_pool(name="kpool", bufs=1) as kpoolp,
            tc.tile_pool(name="vpool", bufs=1) as vpoolp,
            tc.tile_pool(name="xpool", bufs=1) as xpoolp,
            # transient rings
            tc.tile_pool(name="qpool", bufs=6, side="right") as qpoolp,
            tc.tile_pool(name="maskp", bufs=3, side="right") as maskp,
            tc.tile_pool(name="ptp", bufs=2, side="right") as ptp,
            tc.tile_pool(name="cup", bufs=3, side="right") as cup,
            tc.tile_pool(name="pps", space="PSUM", bufs=ppsbufs) as ppsp,
            tc.tile_pool(name="miscps", space="PSUM", bufs=1) as miscp,
            tc.tile_pool(name="projps", space="PSUM", bufs=1) as projp,
        ):
            # ---------------- persistent tiles ----------------
            if qk8:
                wqk_sb = constp.tile([128, 4 * NDP * 2 * 128], F8)
                nc.sync.dma_start(
                    wqk_sb.rearrange("p (b c t m) -> p b c t m",
                                     b=4, c=NDP, t=2),
                    wqk.rearrange("(c t p) (b m) -> p b c t m",
                                  p=128, t=2, b=4))
            else:
                wqk_sb = constp.tile([128, ND * 512], BF16)
                nc.sync.dma_start(
                    wqk_sb.rearrange("p (d c) -> p d c", d=ND),
                    wqkb.rearrange("(d p) c -> p d c", p=128))
            wv_sb = constp.tile([128, ND * 256], BF16)
            nc.sync.dma_start(
                wv_sb.rearrange("p (d c) -> p d c", d=ND),
                wv.rearrange("(d p) c -> p d c", p=128))
            wo_sb = constp.tile([Dh + 1, HL * (Dh + 1)], BF16)
            nc.sync.dma_start(wo_sb, wo)
            bqk_sb = constp.tile([128, 4], F32)
            nc.sync.dma_start(bqk_sb, bqk)
            bv_sb = constp.tile([128, HL * Dh], F32)
            nc.sync.dma_start(bv_sb, bv)

            # kT per head pair g: rows 0-63 head 2g, rows 64-127 head 2g+1
            kt = [kpoolp.tile([128, S], BF16, name=f"kt{g}")
                  for g in range(HL // 2)]
            # v in [sk, d] layout + fused ones column per head
            v_sb = vpoolp.tile([128, NCH * HL * 65], BF16)
            nc.vector.memset(
                v_sb.rearrange("p (m c) -> p m c", c=65)[:, :, 64:65], 1.0)
            # x^T tiles, one per 512-column block (= per query group)
            xt = [xpoolp.tile([128, ND * 512], BF16, name=f"xt{s4}")
                  for s4 in range(NQG)]
            xt8 = [xpoolp.tile([128, NDP * 2 * 512], F8, name=f"xt8{s4}")
                   for s4 in range(NQG)] if qk8 else None

            def dma_x(s4):
                if "dma_x" in ablate:
                    return
                nc.sync.dma_start(
                    xt[s4].rearrange("p (d c) -> p d c", d=ND),
                    xT.rearrange("(d p) s -> p d s", p=128)
                      [:, :, s4 * 512:(s4 + 1) * 512])
                if qk8:
                    nc.sync.dma_start(
                        xt8[s4].rearrange("p (c t s) -> p c t s",
                                          c=NDP, t=2),
                        xT8.rearrange("(c t p) s -> p c t s", p=128, t=2)
                           [:, :, :, s4 * 512:(s4 + 1) * 512])

            def kproj(s4, use_misc):
                """k-proj for x block s4: both pair-blocks -> kt slices."""
                if qk8:
                    wv8 = wqk_sb.rearrange("p (b c t m) -> p b c t m",
                                           b=4, c=NDP, t=2)
                    xv8 = xt8[s4].rearrange("p (c t s) -> p c t s",
                                            c=NDP, t=2)
                for blk in range(2):
                    pool = miscp if (use_misc ^ (blk == 0)) else projp
                    ps = pool.tile([128, 512], F32, tag="mc" if pool is miscp
                                   else "pj")
                    if qk8:
                        nlast = NDP - 1 if "qkv" not in ablate else 0
                        for c in range(nlast + 1):
                            nc.tensor.matmul(
                                ps,
                                lhsT=wv8[:, 2 + blk, c],
                                rhs=xv8[:, c],
                                start=(c == 0), stop=(c == nlast),
                                perf_mode=DR)
                    else:
                        nlast = ND - 1 if "qkv" not in ablate else 0
                        for dd in range(nlast + 1):
                            nc.tensor.matmul(
                                ps,
                                lhsT=wqk_sb[:, dd * 512 + (2 + blk) * 128:
                                            dd * 512 + (3 + blk) * 128],
                                rhs=xt[s4][:, dd * 512:(dd + 1) * 512],
                                start=(dd == 0), stop=(dd == nlast))
                    nc.vector.tensor_scalar_add(
                        kt[blk][:, s4 * 512:(s4 + 1) * 512], ps,
                        bqk_sb[:, 2 + blk:3 + blk])

            def qproj(qg):
                """q-proj for query group qg -> fresh qt tiles (per pair)."""
                qts = []
                if qk8:
                    wv8 = wqk_sb.rearrange("p (b c t m) -> p b c t m",
                                           b=4, c=NDP, t=2)
                    xv8 = xt8[qg].rearrange("p (c t s) -> p c t s",
                                            c=NDP, t=2)
                for blk in range(2):
                    ps = projp.tile([128, 512], F32, tag="pj")
                    if qk8:
                        nlast = NDP - 1 if "qkv" not in ablate else 0
                        for c in range(nlast + 1):
                            nc.tensor.matmul(
                                ps,
                                lhsT=wv8[:, blk, c],
                                rhs=xv8[:, c],
                                start=(c == 0), stop=(c == nlast),
                                perf_mode=DR)
                    else:
                        nlast = ND - 1 if "qkv" not in ablate else 0
                        for dd in range(nlast + 1):
                            nc.tensor.matmul(
                                ps,
                                lhsT=wqk_sb[:, dd * 512 + blk * 128:
                                            dd * 512 + (blk + 1) * 128],
                                rhs=xt[qg][:, dd * 512:(dd + 1) * 512],
                                start=(dd == 0), stop=(dd == nlast))
                    qt = qpoolp.tile([128, 512], BF16, tag="qt")
                    nc.vector.tensor_scalar_add(
                        qt, ps, bqk_sb[:, blk:blk + 1])
                    qts.append(qt)
                return qts

            def vproj_one(s4, jj):
                """v-proj for one sk chunk (j = s4*4 + jj) -> v_sb."""
                j = s4 * 4 + jj
                pool = miscp if j % 2 == 0 else projp
                psv = pool.tile([128, 512], F32,
                                tag="mc" if pool is miscp else "pj")
                for d in range(ND if "qkv" not in ablate else 1):
                    nc.tensor.matmul(
                        psv[:, 0:256],
                        lhsT=xt[s4][:, d * 512 + jj * 128:
                                    d * 512 + jj * 128 + 128],
                        rhs=wv_sb[:, d * 256:(d + 1) * 256],
                        start=(d == 0),
                        stop=(d == (ND - 1 if "qkv" not in ablate else 0)))
                nc.vector.tensor_add(
                    v_sb[:, j * (HL * 65):(j + 1) * (HL * 65)]
                        .rearrange("p (h c) -> p h c", h=HL)[:, :, 0:64],
                    psv[:, 0:256].rearrange("p (h c) -> p h c", h=HL),
                    bv_sb.rearrange("p (h c) -> p h c", h=HL))

            def dma_mask(qg):
                mts = []
                for r in range(2):
                    mt = maskp.tile([128, 8 * 512], BF16, name=f"mt{r}",
                                    tag=f"mask{r}")
                    if "dma_mask" not in ablate:
                        nc.sync.dma_start(
                            mt.rearrange("p (j c) -> p j c", j=8),
                            maskT.rearrange("(j p) q -> p j q", p=128)
                                 [:, 8 * r:8 * r + 8,
                                  qg * 512:(qg + 1) * 512])
                    mts.append(mt)
                return mts

            def logits(qg, g, qts, filler=None):
                """Logits + exp for head pair g of query group qg.
                Returns the pt tile [128, NSL*512] of unnormalized
                probabilities (mask applied separately).  `filler` is an
                iterator of callables emitting independent PE work between
                pps groups (the pps ring paces PE to ACT speed, so filler
                work runs in the gaps instead of stalling the PE stream)."""
                pt = ptp.tile([128, NSL * 512], BF16, tag="pt")
                s0 = 0
                pps = None
                for j in range(NCH):
                    for hh in range(2):
                        s = 2 * j + hh
                        if pps is None:
                            pps = ppsp.tile([128, gw * 512], F32, tag="pps")
                            s0 = s
                        if "logits" not in ablate or j == 0:
                            nc.tensor.matmul(
                                pps[:, (s - s0) * 512:(s - s0 + 1) * 512],
                                lhsT=kt[g][hh * 64:(hh + 1) * 64,
                                           j * 128:(j + 1) * 128],
                                rhs=qts[g][hh * 64:(hh + 1) * 64, :],
                                start=True, stop=True)
                        if s - s0 == gw - 1 or s == NSL - 1:
                            if "exp" not in ablate or s0 == 0:
                                nc.scalar.activation(
                                    pt[:, s0 * 512:(s + 1) * 512],
                                    pps[:, 0:(s - s0 + 1) * 512], EXP,
                                    scale=SCALE / (W8SCALE * W8SCALE))
                            pps = None
                            if filler is not None:
                                for fn in next(filler, []):
                                    fn()
                return pt

            def mask_mul(pt, mts, r):
                if "mask" in ablate:
                    return
                for jh in range(2):
                    o = (r * 16 + jh * 8) * 512
                    ptv = pt[:, o:o + 8 * 512].rearrange(
                        "p (j e c) -> p j e c", j=4, e=2)
                    mtv = mts[r].rearrange("p (j c) -> p j c", j=8)[
                        :, jh * 4:(jh + 1) * 4, :]
                    for e in range(2):
                        nc.vector.tensor_mul(ptv[:, :, e, :],
                                             ptv[:, :, e, :], mtv)

            def ctx_chunks(h, ctx, pt, hh, j0, j1):
                nj = NCH if "ctx" not in ablate else 1
                for j in range(j0, min(j1, nj)):
                    nc.tensor.matmul(
                        ctx,
                        lhsT=v_sb[:, j * (HL * 65) + h * 65:
                                  j * (HL * 65) + (h + 1) * 65],
                        rhs=pt[:, (2 * j + hh) * 512:
                               (2 * j + hh + 1) * 512],
                        start=(j == 0),
                        stop=(j == nj - 1),
                        skip_group_check=True)

            def ctx_proj_items(rep, qg, g, pt):
                """Generator of filler items: ctx (with fused denominator
                row) + output proj for both heads of pair g, sliced into
                small PE bursts so they interleave with logits groups."""
                for hh in range(2):
                    h = 2 * g + hh
                    ctx = miscp.tile([65, 512], F32, tag="mc")
                    for j0 in range(0, NCH, 4):
                        yield [lambda a=j0: ctx_chunks(h, ctx, pt, hh, a,
                                                       a + 4)]

                    def finish(h=h, ctx=ctx):
                        cu = cup.tile([65, 512], BF16, tag="cu")
                        nc.vector.tensor_copy(cu, ctx)
                        po = miscp.tile([65, 512], F32, tag="mc")
                        if "proj" not in ablate:
                            nc.tensor.matmul(
                                po,
                                lhsT=wo_sb[:, h * 65:(h + 1) * 65],
                                rhs=cu,
                                start=True, stop=True)
                        po_sb = cup.tile([65, 512], BF16, tag="po")
                        nc.vector.tensor_copy(po_sb, po)
                        nc.sync.dma_start(
                            outp[(rep % nrout) * HL + h]
                                [:, qg * 512:(qg + 1) * 512],
                            po_sb)
                    yield [finish]

            def ctx_proj(rep, qg, g, pt):
                for items in ctx_proj_items(rep, qg, g, pt):
                    for fn in items:
                        fn()

            # ---------------- schedule ----------------
            def vproj_groups():
                """One v-proj accumulation group (one sk chunk) at a time,
                ping-ponged across the two spare PSUM banks."""
                for s4 in range(NQG):
                    for jj in range(4):
                        yield [lambda s4=s4, jj=jj: vproj_one(s4, jj)]

            for rep in range(reps):
                for s4 in range(NQG):
                    dma_x(s4)
                # head start: k proj; then attention with v-proj slotted
                # into the pps-ring gaps of the first logits call.
                for s4 in range(NQG):
                    kproj(s4, use_misc=(s4 % 2 == 0))
                mts = dma_mask(0)
                qts = qproj(0)
                vfill = vproj_groups()
                pt00 = logits(0, 0, qts, filler=vfill)
                mask_mul(pt00, mts, 0)
                pt01 = logits(0, 1, qts, filler=vfill)
                mask_mul(pt00, mts, 1)
                qts_n = qproj(1)

                pts = [pt00, pt01]
                for qg in range(NQG):
                    # pt tiles of current qg are in pts; qts_n = q of qg+1
                    mts_n = dma_mask(qg + 1) if qg + 1 < NQG else None
                    if qg + 1 < NQG:
                        # ctx/proj of (qg, g) interleave as filler inside
                        # logits(qg+1, g) so the pps ring keeps ACT fed.
                        fill0 = ctx_proj_items(rep, qg, 0, pts[0])
                        n0 = logits(qg + 1, 0, qts_n, filler=fill0)
                        for items in fill0:
                            for fn in items:
                                fn()
                        mask_mul(pts[1], mts, 0)
                        mask_mul(pts[1], mts, 1)

                        qts_nn = []

                        def fill1_gen(qg=qg, pts=pts, qts_nn=qts_nn):
                            yield from ctx_proj_items(rep, qg, 1, pts[1])
                            if qg + 2 < NQG:
                                yield [lambda: qts_nn.extend(qproj(qg + 2))]

                        fill1 = fill1_gen()
                        n1 = logits(qg + 1, 1, qts_n, filler=fill1)
                        for items in fill1:
                            for fn in items:
                                fn()
                        mask_mul(n0, mts_n, 0)
                        mask_mul(n0, mts_n, 1)
                        pts = [n0, n1]
                        mts = mts_n
                        if qts_nn:
                            qts_n = qts_nn
                    else:
                        ctx_proj(rep, qg, 0, pts[0])
                        mask_mul(pts[1], mts, 0)
                        mask_mul(pts[1], mts, 1)
                        ctx_proj(rep, qg, 1, pts[1])

    nc.compile()
    return nc


_NC_CACHE = {}


def get_module(reps=1, timing_mode=False, ablate=(), qk8=False,
               gw=2, ppsbufs=3):
    key = (reps, timing_mode, tuple(sorted(ablate)), qk8, gw, ppsbufs)
    if key not in _NC_CACHE:
        _NC_CACHE[key] = build_module(reps, ablate=ablate,
                                      timing_mode=timing_mode, qk8=qk8,
                                      gw=gw, ppsbufs=ppsbufs)
    return _NC_CACHE[key]


def make_in_maps(x, W_qkv, b_qkv, W_o, b_o, mask):
    x = np.asarray(x, np.float32)
    W_qkv = np.asarray(W_qkv, np.float32)
    b_qkv = np.asarray(b_qkv, np.float32)
    W_o = np.asarray(W_o, np.float32)
    mask = np.asarray(mask)
    BF = ml_dtypes.bfloat16

    # reference layout: W_qkv[:, h*3*Dh + {0..Dh | Dh..2Dh | 2Dh..3Dh}] =
    # q|k|v of head h (qkv.reshape(B,S,H,3*Dh) then split on last axis)
    W3 = W_qkv.reshape(D, H, 3 * Dh)
    b3 = b_qkv.reshape(H, 3 * Dh)
    Wq = np.ascontiguousarray(W3[:, :, :Dh].reshape(D, H * Dh))
    Wk = np.ascontiguousarray(W3[:, :, Dh:2 * Dh].reshape(D, H * Dh))
    Wv = np.ascontiguousarray(W3[:, :, 2 * Dh:].reshape(D, H * Dh))
    bq = np.ascontiguousarray(b3[:, :Dh].reshape(H * Dh))
    bk = np.ascontiguousarray(b3[:, Dh:2 * Dh].reshape(H * Dh))
    bv_full = np.ascontiguousarray(b3[:, 2 * Dh:].reshape(H * Dh))

    F8NP = ml_dtypes.float8_e4m3
    xT_b = [np.ascontiguousarray(x[b].T).astype(BF) for b in range(B)]
    xT8_b = [np.ascontiguousarray(x[b].T).astype(F8NP) for b in range(B)]
    maskT_b = [np.ascontiguousarray(
        (mask[b, 0] != 0).T.astype(BF)) for b in range(B)]

    in_maps = []
    for c in range(NCORE):
        b = c // GPB
        g0 = (c % GPB) * HL  # first global head of this core
        # pair-blocks: [q(2g0..), q(..), k(..), k(..)] each 128 cols.
        # fp8 path: x16 premultiplier keeps W out of fp8 subnormals; the
        # 1/sqrt(D) scale and the 16*16 factor are folded into exp's scale.
        qcols = [Wq[:, (g0 + 2 * p) * 64:(g0 + 2 * p + 2) * 64] * W8SCALE
                 for p in range(HL // 2)]
        kcols = [Wk[:, (g0 + 2 * p) * 64:(g0 + 2 * p + 2) * 64] * W8SCALE
                 for p in range(HL // 2)]
        wqk_c = np.ascontiguousarray(np.concatenate(qcols + kcols, axis=1))
        wv_c = np.ascontiguousarray(Wv[:, g0 * 64:(g0 + HL) * 64])
        bqk_c = np.stack(
            [bq[(g0 + 2 * p) * 64:(g0 + 2 * p + 2) * 64] * W8SCALE
             for p in range(HL // 2)]
            + [bk[(g0 + 2 * p) * 64:(g0 + 2 * p + 2) * 64] * W8SCALE
               for p in range(HL // 2)], axis=1)
        bv_c = np.tile(bv_full[g0 * 64:(g0 + HL) * 64], (128, 1))
        # augmented per-head proj: [65, 65] with denominator pass-through
        wo_c = np.zeros((Dh + 1, HL * (Dh + 1)), np.float32)
        for h in range(HL):
            wo_c[0:Dh, h * 65:h * 65 + Dh] = W_o[(g0 + h) * 64:
                                                 (g0 + h + 1) * 64, :]
            wo_c[Dh, h * 65 + Dh] = 1.0
        in_maps.append({
            "xT": xT_b[b],
            "xT8": xT8_b[b],
            "wqk": wqk_c.astype(F8NP),
            "wqkb": wqk_c.astype(BF),
            "wv": wv_c.astype(BF),
            "bqk": np.ascontiguousarray(bqk_c, dtype=np.float32),
            "bv": np.ascontiguousarray(bv_c, dtype=np.float32),
            "wo": wo_c.astype(BF),
            "maskT": maskT_b[b],
        })
    return in_maps


def combine_outputs(results, b_o):
    """results: list of 8 dicts with 'outp' [HL, Dh+1, S] (bf16)."""
    b_o = np.asarray(b_o, np.float32)
    out = np.zeros((B, S, Dh), np.float32)
    for c in range(NCORE):
        b = c // GPB
        op = results[c]["outp"].astype(np.float32)     # [HL, 65, S]
        contrib = (op[:, :Dh, :] / op[:, Dh:Dh + 1, :]).sum(axis=0)
        out[b] += contrib.T
    out += b_o[None, None, :]
    return out


def kernel(x, W_qkv, b_qkv, W_o, b_o, mask):
    nc = get_module()
    in_maps = make_in_maps(x, W_qkv, b_qkv, W_o, b_o, mask)
    res = run_bass_kernel_spmd(nc, in_maps, core_ids=list(range(NCORE)))
    return combine_outputs(res.results, b_o)
